# revision 3
# baseline (speedup 1.0000x reference)
"""Trainium2 Bass kernel for nn_CueWordSelectNet (2-layer LSTM + MLP + batch-softmax).

Strategy (8 NeuronCores, SPMD):
  - Hidden/gate dimension sharded 8 ways: core k owns hidden rows [128k, 128k+128)
    of both LSTMs (padded 1000->1024). Weights are replicated-sliced per core and
    stay resident in SBUF across all 64 timesteps.
  - Everything lives in "hT layout": [hidden -> partitions, batch -> free dim],
    so the recurrent matmuls are out[gate,batch] = W[K=hid,M=gate].T @ hT[K,N=batch]
    with N=256 (full batch) and no transposes anywhere.
  - Algebraic fusion: gates1 = x_t@A1.T + h1@B1.T with B1 = W_ih1[:,600:]+W_hh1;
    gates2 = h1@A2.T + h2@B2.T with A2 = W_ih2[:,600:1600], B2 = W_ih2[:,1600:]+W_hh2
    (the zero block of encoder2's input is dropped). This cuts FLOPs ~35%.
  - Per step, each core computes its 512 gate rows (4 gates x 128 hidden), applies
    the LSTM cell math (fp32 PSUM + fp32 c state, fp16 h), and the 8 h-chunks are
    recombined with an AllGather. The loop is restructured so iteration t computes
    h2(t) then h1(t+1): two AllGathers per iteration that pipeline with compute.
  - Head: mid = [h1;topic]@W1.T+b1 sharded by mid rows, then out partials
    W2-chunk.T@mid-chunk ReduceScattered (add) so each core lands exactly its
    output rows; softmax over the batch (free) dim is local per core.
  - Matmul dtype fp16 (1 cycle/row on PE, ~10x tighter than bf16; fp32 accumulate).

Host side only shards/pads/transposes inputs and reassembles outputs.
"""
import numpy as np

import concourse.bacc as bacc
import concourse.mybir as mybir
import concourse.tile as tile
from concourse import bass_utils

F16 = mybir.dt.float16
F32 = mybir.dt.float32
AF = mybir.ActivationFunctionType
ALU = mybir.AluOpType

N_CORES = 8
B = 256          # batch (free dim everywhere)
T = 64
D, Dp = 600, 640         # input size, padded (5 k-tiles)
H, Hp = 1000, 1024       # hidden, padded (8 k-tiles)
GC = 512                 # per-core gate rows (4 gates x 128)
MID, MIDp = 4000, 4096   # head inner dim, padded (512 per core)
OUT, OUTp = 1000, 1024   # head output dim, padded (128 per core)

KH = Hp // 128   # 8 hidden k-tiles
KX = Dp // 128   # 5 x k-tiles

_CACHE = {}


def _build_nc(n_steps=T):
    nc = bacc.Bacc("TRN2", target_bir_lowering=False, debug=False, num_devices=N_CORES)

    # ---- per-core external inputs (all pre-laid-out on host) ----
    din = {}
    din["xdev"] = nc.dram_tensor("xdev", [n_steps, 128, KX * B], F16, kind="ExternalInput").ap()
    din["topict"] = nc.dram_tensor("topict", [128, KH * B], F16, kind="ExternalInput").ap()
    for name, cols in [("a1w", KX * GC), ("b1w", KH * GC), ("a2w", KH * GC),
                       ("b2w", KH * GC), ("w1w", 2 * KH * GC), ("w2w", 4 * OUTp)]:
        din[name] = nc.dram_tensor(name, [128, cols], F16, kind="ExternalInput").ap()
    for name, cols in [("b1c", 4), ("b2c", 4), ("bh1", 4), ("bh2", 1)]:
        din[name] = nc.dram_tensor(name, [128, cols], F32, kind="ExternalInput").ap()

    # ---- per-core external outputs ----
    sm_out = nc.dram_tensor("sm", [128, B], F32, kind="ExternalOutput").ap()
    h1_out = nc.dram_tensor("h1c", [128, B], F16, kind="ExternalOutput").ap()
    c1_out = nc.dram_tensor("c1c", [128, B], F32, kind="ExternalOutput").ap()
    h2_out = nc.dram_tensor("h2c", [128, B], F16, kind="ExternalOutput").ap()
    c2_out = nc.dram_tensor("c2c", [128, B], F32, kind="ExternalOutput").ap()

    with tile.TileContext(nc) as tc:
        with (
            tc.tile_pool(name="wp", bufs=1) as wp,          # persistent weights
            tc.tile_pool(name="hp", bufs=2) as hp,          # gathered h states
            tc.tile_pool(name="xp", bufs=3) as xp,          # x_t prefetch
            tc.tile_pool(name="gm", bufs=3) as gm,          # gate-math temporaries
            tc.tile_pool(name="cs", bufs=2) as cs,          # c states
            tc.tile_pool(name="ps", bufs=1, space="PSUM") as ps,
            tc.tile_pool(name="dr", bufs=2, space="DRAM") as dr,
        ):
            # ---- load resident weights/biases ----
            w = {}
            for name in ("a1w", "b1w", "a2w", "b2w", "w1w", "w2w"):
                cols = din[name].shape[-1]
                w[name] = wp.tile([128, cols], F16, name=f"sb_{name}", tag=name)
                nc.sync.dma_start(w[name][:], din[name][:])
            bias = {}
            for name in ("b1c", "b2c", "bh1", "bh2"):
                cols = din[name].shape[-1]
                bias[name] = wp.tile([128, cols], F32, name=f"sb_{name}", tag=name)
                nc.sync.dma_start(bias[name][:], din[name][:])
            topict = wp.tile([128, KH * B], F16, tag="topict")
            nc.sync.dma_start(topict[:], din["topict"][:])

            # psum banks: 4 for gates1, 4 for gates2 (reused by the head)
            def psum_banks(prefix):
                return [ps.tile([128, B], F32, name=f"{prefix}{m}", tag=f"{prefix}{m}")
                        for m in range(4)]

            def load_x(t):
                xt = xp.tile([128, KX * B], F16, name=f"x_{t}", tag="xt")
                nc.sync.dma_start(xt[:], din["xdev"][t])
                return xt

            def gate_math(pb, bias_ap, c_prev, first, idx):
                """LSTM cell math from 4 psum banks; returns (h_chunk f16, c_new f32)."""
                si = gm.tile([128, B], F32, name=f"si_{idx}", tag="si")
                tg = gm.tile([128, B], F32, name=f"tg_{idx}", tag="tg")
                so = gm.tile([128, B], F32, name=f"so_{idx}", tag="so")
                nc.scalar.activation(si[:], pb[0][:], AF.Sigmoid, bias=bias_ap[:, 0:1])
                nc.scalar.activation(tg[:], pb[2][:], AF.Tanh, bias=bias_ap[:, 2:3])
                nc.scalar.activation(so[:], pb[3][:], AF.Sigmoid, bias=bias_ap[:, 3:4])
                cn = cs.tile([128, B], F32, name=f"c_{idx}", tag=f"c{idx[0]}")
                if first:
                    nc.vector.tensor_tensor(cn[:], si[:], tg[:], ALU.mult)
                else:
                    sf = gm.tile([128, B], F32, name=f"sf_{idx}", tag="sf")
                    nc.scalar.activation(sf[:], pb[1][:], AF.Sigmoid, bias=bias_ap[:, 1:2])
                    t1 = gm.tile([128, B], F32, name=f"t1_{idx}", tag="t1")
                    nc.vector.tensor_tensor(t1[:], sf[:], c_prev[:], ALU.mult)
                    t2 = gm.tile([128, B], F32, name=f"t2_{idx}", tag="t2")
                    nc.vector.tensor_tensor(t2[:], si[:], tg[:], ALU.mult)
                    nc.vector.tensor_tensor(cn[:], t1[:], t2[:], ALU.add)
                tcn = gm.tile([128, B], F32, name=f"tcn_{idx}", tag="tcn")
                nc.scalar.activation(tcn[:], cn[:], AF.Tanh)
                hch = gm.tile([128, B], F16, name=f"h_{idx}", tag=f"hch{idx[0]}")
                nc.vector.tensor_tensor(hch[:], so[:], tcn[:], ALU.mult)
                return hch, cn

            def gather(hch, idx):
                """AllGather h chunk -> full hT [128, KH*B] in SBUF."""
                bin_ = dr.tile([128, B], F16, name=f"bin_{idx}", tag=f"bin{idx[0]}")
                bout = dr.tile([Hp, B], F16, name=f"bout_{idx}", tag=f"bout{idx[0]}")
                nc.sync.dma_start(bin_[:], hch[:])
                nc.gpsimd.collective_compute(
                    "AllGather", ALU.bypass,
                    replica_groups=[list(range(N_CORES))],
                    ins=[bin_.opt()], outs=[bout.opt()])
                hT = hp.tile([128, KH * B], F16, name=f"hT_{idx}", tag=f"hT{idx[0]}")
                nc.sync.dma_start(hT.rearrange("p (k n) -> p k n", n=B),
                                  bout.rearrange("(k p) n -> p k n", p=128))
                return hT

            def mm_group(pb, wt, rhs, ktiles, woff, start, stop):
                """Accumulate ktiles matmuls into the 4 psum banks."""
                for m in range(4):
                    for k in range(ktiles):
                        nc.tensor.matmul(
                            pb[m][:],
                            wt[:, woff + k * GC + m * 128: woff + k * GC + (m + 1) * 128],
                            rhs[:, k * B:(k + 1) * B],
                            start=(start and k == 0),
                            stop=(stop and k == ktiles - 1))

            # ================= prologue: t = 0 =================
            x0 = load_x(0)
            g1 = psum_banks("g1")
            mm_group(g1, w["a1w"], x0, KX, 0, start=True, stop=True)
            h1ch, c1 = gate_math(g1, bias["b1c"], None, True, ("1", 0))
            h1T = gather(h1ch, ("1", 0))

            h2T, c2, h2ch = None, None, None
            x_next = load_x(1) if n_steps > 1 else None

            # ================= main loop =================
            # iteration t: computes h2(t) chunk, then h1(t+1) chunk
            for t in range(n_steps - 1):
                with nc.named_scope(f"step{t}"):
                    g2 = psum_banks("g2")
                    first2 = (t == 0)
                    if not first2:
                        mm_group(g2, w["b2w"], h2T, KH, 0, start=True, stop=False)
                    g1 = psum_banks("g1")
                    mm_group(g1, w["a1w"], x_next, KX, 0, start=True, stop=False)
                    mm_group(g2, w["a2w"], h1T, KH, 0, start=first2, stop=True)
                    h2ch, c2 = gate_math(g2, bias["b2c"], c2, first2, ("2", t))
                    h2T = gather(h2ch, ("2", t))
                    if t + 2 < n_steps:
                        x_next = load_x(t + 2)
                    mm_group(g1, w["b1w"], h1T, KH, 0, start=False, stop=True)
                    h1ch, c1 = gate_math(g1, bias["b1c"], c1, False, ("1", t + 1))
                    h1T = gather(h1ch, ("1", t + 1))

            # ================= epilogue: gates2(T-1) =================
            with nc.named_scope("epilogue"):
                g2 = psum_banks("g2")
                if n_steps > 1:
                    mm_group(g2, w["b2w"], h2T, KH, 0, start=True, stop=False)
                    mm_group(g2, w["a2w"], h1T, KH, 0, start=False, stop=True)
                    h2ch, c2 = gate_math(g2, bias["b2c"], c2, False, ("2", n_steps - 1))
                else:
                    mm_group(g2, w["a2w"], h1T, KH, 0, start=True, stop=True)
                    h2ch, c2 = gate_math(g2, bias["b2c"], None, True, ("2", 0))

            # state outputs
            nc.sync.dma_start(h1_out[:], h1ch[:])
            nc.sync.dma_start(c1_out[:], c1[:])
            nc.sync.dma_start(h2_out[:], h2ch[:])
            nc.sync.dma_start(c2_out[:], c2[:])

            # ================= head =================
            with nc.named_scope("head"):
                # mid chunk [512,B] = W1c.T @ [h1T; topicT] + bh1  -> f16
                midp = psum_banks("g1")
                mm_group(midp, w["w1w"], h1T, KH, 0, start=True, stop=False)
                mm_group(midp, w["w1w"], topict, KH, KH * GC, start=False, stop=True)
                mid16 = gm.tile([128, 4 * B], F16, tag="mid16")
                for m in range(4):
                    nc.vector.tensor_scalar_add(
                        mid16[:, m * B:(m + 1) * B], midp[m][:], bias["bh1"][:, m:m + 1])

                # partial outT [1024,B] f32 over this core's mid chunk
                pof32 = gm.tile([128, 8 * B], F32, tag="pof32")
                for m2 in range(8):
                    pb = ps.tile([128, B], F32, name=f"po_{m2}",
                                 tag=f"g{1 + m2 // 4}{m2 % 4}")
                    for k in range(4):
                        nc.tensor.matmul(
                            pb[:],
                            w["w2w"][:, k * OUTp + m2 * 128: k * OUTp + (m2 + 1) * 128],
                            mid16[:, k * B:(k + 1) * B],
                            start=(k == 0), stop=(k == 3))
                    nc.vector.tensor_copy(pof32[:, m2 * B:(m2 + 1) * B], pb[:])

                rsin = dr.tile([OUTp, B], F32, tag="rsin")
                nc.sync.dma_start(rsin.rearrange("(k p) n -> p k n", p=128),
                                  pof32.rearrange("p (k n) -> p k n", n=B))
                rsout = dr.tile([128, B], F32, tag="rsout")
                nc.gpsimd.collective_compute(
                    "ReduceScatter", ALU.add,
                    replica_groups=[list(range(N_CORES))],
                    ins=[rsin.opt()], outs=[rsout.opt()])

                mb = gm.tile([128, B], F32, tag="mb")
                nc.sync.dma_start(mb[:], rsout[:])
                m2b = gm.tile([128, B], F32, tag="m2b")
                nc.vector.tensor_scalar_add(m2b[:], mb[:], bias["bh2"][:, 0:1])
                negmax = gm.tile([128, 1], F32, tag="negmax")
                nc.vector.reduce_max(negmax[:], m2b[:], axis=mybir.AxisListType.X, negate=True)
                ex = gm.tile([128, B], F32, tag="ex")
                ssum = gm.tile([128, 1], F32, tag="ssum")
                nc.scalar.activation(ex[:], m2b[:], AF.Exp, bias=negmax[:], accum_out=ssum[:])
                rec = gm.tile([128, 1], F32, tag="rec")
                nc.vector.reciprocal(rec[:], ssum[:])
                smt = gm.tile([128, B], F32, tag="smt")
                nc.vector.tensor_scalar_mul(smt[:], ex[:], rec[:])
                nc.sync.dma_start(sm_out[:], smt[:])

    nc.compile()
    return nc


def _sbuf_layout(lhsT):
    """[K, M] -> [128, (K/128)*M] host layout (k-tiles side by side)."""
    K, M = lhsT.shape
    assert K % 128 == 0
    return np.ascontiguousarray(
        lhsT.reshape(K // 128, 128, M).transpose(1, 0, 2).reshape(128, (K // 128) * M))


def _prep_inputs(inputs):
    f32 = np.float32
    W_ih1 = np.asarray(inputs["W_ih1"], f32); W_hh1 = np.asarray(inputs["W_hh1"], f32)
    W_ih2 = np.asarray(inputs["W_ih2"], f32); W_hh2 = np.asarray(inputs["W_hh2"], f32)
    W1 = np.asarray(inputs["W1"], f32); W2 = np.asarray(inputs["W2"], f32)
    b1 = np.asarray(inputs["b_ih1"], f32) + np.asarray(inputs["b_hh1"], f32)
    b2 = np.asarray(inputs["b_ih2"], f32) + np.asarray(inputs["b_hh2"], f32)
    bw1 = np.asarray(inputs["b1"], f32); bw2 = np.asarray(inputs["b2"], f32)
    X = np.asarray(inputs["input"], f32)          # (B, T, D)
    hidx = np.asarray(inputs["h"])                # (B, 5) int

    def pad_gate(Wm, kin, kin_p):
        """(4000, kin) -> (4096, kin_p) in padded gate space."""
        out = np.zeros((4, Hp, kin_p), f32)
        out[:, :H, :kin] = Wm.reshape(4, H, kin)
        return out.reshape(4 * Hp, kin_p)

    A1 = pad_gate(W_ih1[:, :D], D, Dp)
    B1 = pad_gate(W_ih1[:, D:] + W_hh1, H, Hp)
    A2 = pad_gate(W_ih2[:, D:D + H], H, Hp)
    B2 = pad_gate(W_ih2[:, D + H:] + W_hh2, H, Hp)

    # W1 in per-core-chunk padded mid space; columns [h1 | topic] each padded
    W1p = np.zeros((MIDp, 2 * Hp), f32)
    for k in range(N_CORES):
        W1p[k * 512:k * 512 + 500, :H] = W1[k * 500:(k + 1) * 500, :H]
        W1p[k * 512:k * 512 + 500, Hp:Hp + H] = W1[k * 500:(k + 1) * 500, H:]
    W2p = np.zeros((OUTp, MIDp), f32)
    for k in range(N_CORES):
        W2p[:OUT, k * 512:k * 512 + 500] = W2[:, k * 500:(k + 1) * 500]

    b1p = np.zeros((4, Hp), f32); b1p[:, :H] = b1.reshape(4, H)
    b2p = np.zeros((4, Hp), f32); b2p[:, :H] = b2.reshape(4, H)
    bw1p = np.zeros(MIDp, f32)
    for k in range(N_CORES):
        bw1p[k * 512:k * 512 + 500] = bw1[k * 500:(k + 1) * 500]
    bw2p = np.zeros(OUTp, f32); bw2p[:OUT] = bw2

    # x in device layout: (T, 128, KX*B) f16, replicated
    Xt = np.zeros((T, Dp, B), f32)
    Xt[:, :D, :] = X.transpose(1, 2, 0)
    xdev = np.ascontiguousarray(
        Xt.reshape(T, KX, 128, B).transpose(0, 2, 1, 3).reshape(T, 128, KX * B)
    ).astype(np.float16)

    topic = np.zeros((Hp, B), f32)
    topic[hidx.T % Hp, np.arange(B)[None, :]] = 1.0
    tp = topic.copy(); tp[H:] = 0.0
    topict = _sbuf_layout(tp).astype(np.float16)

    in_maps = []
    for k in range(N_CORES):
        rows = np.concatenate([np.arange(g * Hp + 128 * k, g * Hp + 128 * (k + 1))
                               for g in range(4)])
        d = {
            "xdev": xdev,
            "topict": topict,
            "a1w": _sbuf_layout(np.ascontiguousarray(A1[rows].T)).astype(np.float16),
            "b1w": _sbuf_layout(np.ascontiguousarray(B1[rows].T)).astype(np.float16),
            "a2w": _sbuf_layout(np.ascontiguousarray(A2[rows].T)).astype(np.float16),
            "b2w": _sbuf_layout(np.ascontiguousarray(B2[rows].T)).astype(np.float16),
            "w1w": _sbuf_layout(np.ascontiguousarray(W1p[k * 512:(k + 1) * 512].T)).astype(np.float16),
            "w2w": _sbuf_layout(np.ascontiguousarray(W2p[:, k * 512:(k + 1) * 512].T)).astype(np.float16),
            "b1c": np.ascontiguousarray(b1p[:, 128 * k:128 * (k + 1)].T),
            "b2c": np.ascontiguousarray(b2p[:, 128 * k:128 * (k + 1)].T),
            "bh1": np.ascontiguousarray(bw1p[k * 512:(k + 1) * 512].reshape(4, 128).T),
            "bh2": np.ascontiguousarray(bw2p[128 * k:128 * (k + 1)].reshape(128, 1)),
        }
        in_maps.append(d)
    return in_maps


def kernel(trace=False, tmpdir=None, **inputs):
    if "nc" not in _CACHE:
        _CACHE["nc"] = _build_nc()
    nc = _CACHE["nc"]
    in_maps = _prep_inputs(inputs)
    kw = {}
    if trace:
        kw = dict(trace=True, tmpdir=tmpdir)
    r = bass_utils.run_bass_kernel_spmd(nc, in_maps, core_ids=list(range(N_CORES)), **kw)
    _CACHE["last_results"] = r
    res = r.results

    sm = np.concatenate([res[k]["sm"] for k in range(N_CORES)], axis=0)
    h1 = np.concatenate([res[k]["h1c"] for k in range(N_CORES)], axis=0).astype(np.float32)
    c1 = np.concatenate([res[k]["c1c"] for k in range(N_CORES)], axis=0)
    h2 = np.concatenate([res[k]["h2c"] for k in range(N_CORES)], axis=0).astype(np.float32)
    c2 = np.concatenate([res[k]["c2c"] for k in range(N_CORES)], axis=0)

    softmax = np.ascontiguousarray(sm[:OUT].T)
    return (softmax,
            (np.ascontiguousarray(h1[:H].T), np.ascontiguousarray(c1[:H].T)),
            (np.ascontiguousarray(h2[:H].T), np.ascontiguousarray(c2[:H].T)))


# revision 5
# speedup vs baseline: 1.0080x; 1.0080x over previous
"""Trainium2 Bass kernel for nn_CueWordSelectNet (2-layer LSTM + MLP + batch-softmax).

Strategy (8 NeuronCores, SPMD):
  - Hidden/gate dimension sharded 8 ways: core k owns hidden rows [128k, 128k+128)
    of both LSTMs (padded 1000->1024). Weights are replicated-sliced per core and
    stay resident in SBUF across all 64 timesteps.
  - Everything lives in "hT layout": [hidden -> partitions, batch -> free dim],
    so the recurrent matmuls are out[gate,batch] = W[K=hid,M=gate].T @ hT[K,N=batch]
    with N=256 (full batch) and no transposes anywhere.
  - Algebraic fusion: gates1 = x_t@A1.T + h1@B1.T with B1 = W_ih1[:,600:]+W_hh1;
    gates2 = h1@A2.T + h2@B2.T with A2 = W_ih2[:,600:1600], B2 = W_ih2[:,1600:]+W_hh2
    (the zero block of encoder2's input is dropped). This cuts FLOPs ~35%.
  - Per step, each core computes its 512 gate rows (4 gates x 128 hidden), applies
    the LSTM cell math (fp32 PSUM + fp32 c state, fp16 h), and the 8 h-chunks are
    recombined with an AllGather. The loop is restructured so iteration t computes
    h2(t) then h1(t+1): two AllGathers per iteration that pipeline with compute.
  - Head: mid = [h1;topic]@W1.T+b1 sharded by mid rows, then out partials
    W2-chunk.T@mid-chunk ReduceScattered (add) so each core lands exactly its
    output rows; softmax over the batch (free) dim is local per core.
  - Matmul dtype fp16 (1 cycle/row on PE, ~10x tighter than bf16; fp32 accumulate).

Host side only shards/pads/transposes inputs and reassembles outputs.
"""
import numpy as np

import concourse.bacc as bacc
import concourse.mybir as mybir
import concourse.tile as tile
from concourse import bass_utils

F16 = mybir.dt.float16
F32 = mybir.dt.float32
AF = mybir.ActivationFunctionType
ALU = mybir.AluOpType

N_CORES = 8
B = 256          # batch (free dim everywhere)
T = 64
D, Dp = 600, 640         # input size, padded (5 k-tiles)
H, Hp = 1000, 1024       # hidden, padded (8 k-tiles)
GC = 512                 # per-core gate rows (4 gates x 128)
MID, MIDp = 4000, 4096   # head inner dim, padded (512 per core)
OUT, OUTp = 1000, 1024   # head output dim, padded (128 per core)

KH = Hp // 128   # 8 hidden k-tiles
KX = Dp // 128   # 5 x k-tiles

_CACHE = {}


def _build_nc(n_steps=T):
    nc = bacc.Bacc("TRN2", target_bir_lowering=False, debug=False, num_devices=N_CORES)

    # ---- per-core external inputs (all pre-laid-out on host) ----
    din = {}
    din["xdev"] = nc.dram_tensor("xdev", [n_steps, 128, KX * B], F16, kind="ExternalInput").ap()
    din["topict"] = nc.dram_tensor("topict", [128, KH * B], F16, kind="ExternalInput").ap()
    for name, cols in [("a1w", KX * GC), ("b1w", KH * GC), ("a2w", KH * GC),
                       ("b2w", KH * GC), ("w1w", 2 * KH * GC), ("w2w", 4 * OUTp)]:
        din[name] = nc.dram_tensor(name, [128, cols], F16, kind="ExternalInput").ap()
    for name, cols in [("b1c", 4), ("b2c", 4), ("bh1", 4), ("bh2", 1)]:
        din[name] = nc.dram_tensor(name, [128, cols], F32, kind="ExternalInput").ap()

    # ---- per-core external outputs ----
    sm_out = nc.dram_tensor("sm", [128, B], F32, kind="ExternalOutput").ap()
    h1_out = nc.dram_tensor("h1c", [128, B], F16, kind="ExternalOutput").ap()
    c1_out = nc.dram_tensor("c1c", [128, B], F32, kind="ExternalOutput").ap()
    h2_out = nc.dram_tensor("h2c", [128, B], F16, kind="ExternalOutput").ap()
    c2_out = nc.dram_tensor("c2c", [128, B], F32, kind="ExternalOutput").ap()

    with tile.TileContext(nc) as tc:
        with (
            tc.tile_pool(name="wp", bufs=1) as wp,          # persistent weights
            tc.tile_pool(name="hp", bufs=2) as hp,          # gathered h states
            tc.tile_pool(name="xp", bufs=3) as xp,          # x_t prefetch
            tc.tile_pool(name="gm", bufs=3) as gm,          # gate-math temporaries
            tc.tile_pool(name="cs", bufs=2) as cs,          # c states
            tc.tile_pool(name="ps", bufs=1, space="PSUM") as ps,
            tc.tile_pool(name="dr", bufs=2, space="DRAM") as dr,
        ):
            # ---- load resident weights/biases ----
            w = {}
            for name in ("a1w", "b1w", "a2w", "b2w", "w1w", "w2w"):
                cols = din[name].shape[-1]
                w[name] = wp.tile([128, cols], F16, name=f"sb_{name}", tag=name)
                nc.sync.dma_start(w[name][:], din[name][:])
            bias = {}
            for name in ("b1c", "b2c", "bh1", "bh2"):
                cols = din[name].shape[-1]
                bias[name] = wp.tile([128, cols], F32, name=f"sb_{name}", tag=name)
                nc.sync.dma_start(bias[name][:], din[name][:])
            topict = wp.tile([128, KH * B], F16, tag="topict")
            nc.sync.dma_start(topict[:], din["topict"][:])

            # psum banks: 4 for gates1, 4 for gates2 (reused by the head)
            def psum_banks(prefix):
                return [ps.tile([128, B], F32, name=f"{prefix}{m}", tag=f"{prefix}{m}")
                        for m in range(4)]

            def load_x(t):
                xt = xp.tile([128, KX * B], F16, name=f"x_{t}", tag="xt")
                nc.sync.dma_start(xt[:], din["xdev"][t])
                return xt

            def gate_math(pb, bias_ap, c_prev, first, idx):
                """LSTM cell math from 4 psum banks; returns (h_chunk f16, c_new f32)."""
                si = gm.tile([128, B], F32, name=f"si_{idx}", tag="si")
                tg = gm.tile([128, B], F32, name=f"tg_{idx}", tag="tg")
                so = gm.tile([128, B], F32, name=f"so_{idx}", tag="so")
                nc.scalar.activation(si[:], pb[0][:], AF.Sigmoid, bias=bias_ap[:, 0:1])
                nc.scalar.activation(tg[:], pb[2][:], AF.Tanh, bias=bias_ap[:, 2:3])
                nc.scalar.activation(so[:], pb[3][:], AF.Sigmoid, bias=bias_ap[:, 3:4])
                cn = cs.tile([128, B], F32, name=f"c_{idx}", tag=f"c{idx[0]}")
                if first:
                    nc.vector.tensor_tensor(cn[:], si[:], tg[:], ALU.mult)
                else:
                    sf = gm.tile([128, B], F32, name=f"sf_{idx}", tag="sf")
                    nc.scalar.activation(sf[:], pb[1][:], AF.Sigmoid, bias=bias_ap[:, 1:2])
                    t1 = gm.tile([128, B], F32, name=f"t1_{idx}", tag="t1")
                    nc.vector.tensor_tensor(t1[:], sf[:], c_prev[:], ALU.mult)
                    t2 = gm.tile([128, B], F32, name=f"t2_{idx}", tag="t2")
                    nc.vector.tensor_tensor(t2[:], si[:], tg[:], ALU.mult)
                    nc.vector.tensor_tensor(cn[:], t1[:], t2[:], ALU.add)
                tcn = gm.tile([128, B], F32, name=f"tcn_{idx}", tag="tcn")
                nc.scalar.activation(tcn[:], cn[:], AF.Tanh)
                hch = gm.tile([128, B], F16, name=f"h_{idx}", tag=f"hch{idx[0]}")
                nc.vector.tensor_tensor(hch[:], so[:], tcn[:], ALU.mult)
                return hch, cn

            def gather(hch, idx):
                """AllGather h chunk -> full hT [128, KH*B] in SBUF."""
                bin_ = dr.tile([128, B], F16, name=f"bin_{idx}", tag=f"bin{idx[0]}")
                bout = dr.tile([Hp, B], F16, name=f"bout_{idx}", tag=f"bout{idx[0]}")
                nc.sync.dma_start(bin_[:], hch[:])
                nc.gpsimd.collective_compute(
                    "AllGather", ALU.bypass,
                    replica_groups=[list(range(N_CORES))],
                    ins=[bin_.opt()], outs=[bout.opt()])
                hT = hp.tile([128, KH * B], F16, name=f"hT_{idx}", tag=f"hT{idx[0]}")
                # per-k-tile DMAs so the first k-tile's matmuls start early
                for k in range(KH):
                    nc.sync.dma_start(hT[:, k * B:(k + 1) * B],
                                      bout[128 * k:128 * (k + 1), :])
                return hT

            def mm_group(pb, wt, rhs, ktiles, woff, start, stop):
                """Accumulate ktiles matmuls into the 4 psum banks."""
                for m in range(4):
                    for k in range(ktiles):
                        nc.tensor.matmul(
                            pb[m][:],
                            wt[:, woff + k * GC + m * 128: woff + k * GC + (m + 1) * 128],
                            rhs[:, k * B:(k + 1) * B],
                            start=(start and k == 0),
                            stop=(stop and k == ktiles - 1))

            # ================= prologue: t = 0 =================
            x0 = load_x(0)
            g1 = psum_banks("g1")
            mm_group(g1, w["a1w"], x0, KX, 0, start=True, stop=True)
            h1ch, c1 = gate_math(g1, bias["b1c"], None, True, ("1", 0))
            h1T = gather(h1ch, ("1", 0))

            h2T, c2, h2ch = None, None, None
            x_next = load_x(1) if n_steps > 1 else None

            # ================= main loop =================
            # iteration t: computes h1(t+1) chunk first (critical AG path),
            # then h2(t); A2/B2 parts ride in the AG stall windows.
            for t in range(n_steps - 1):
                with nc.named_scope(f"step{t}"):
                    g2 = psum_banks("g2")
                    first2 = (t == 0)
                    if not first2:
                        # hides in the previous AG1 stall window
                        mm_group(g2, w["b2w"], h2T, KH, 0, start=True, stop=False)
                    g1 = psum_banks("g1")
                    mm_group(g1, w["a1w"], x_next, KX, 0, start=True, stop=False)
                    # first consumer of h1(t): get gm1 -> AG1 going ASAP
                    mm_group(g1, w["b1w"], h1T, KH, 0, start=False, stop=True)
                    h1ch, c1 = gate_math(g1, bias["b1c"], c1, False, ("1", t + 1))
                    h1T_new = gather(h1ch, ("1", t + 1))
                    # g2(t) also uses the OLD h1(t); runs while AG1 is in flight
                    mm_group(g2, w["a2w"], h1T, KH, 0, start=first2, stop=True)
                    h2ch, c2 = gate_math(g2, bias["b2c"], c2, first2, ("2", t))
                    h2T = gather(h2ch, ("2", t))
                    if t + 2 < n_steps:
                        x_next = load_x(t + 2)
                    h1T = h1T_new

            # ================= epilogue: gates2(T-1) =================
            with nc.named_scope("epilogue"):
                g2 = psum_banks("g2")
                if n_steps > 1:
                    mm_group(g2, w["b2w"], h2T, KH, 0, start=True, stop=False)
                    mm_group(g2, w["a2w"], h1T, KH, 0, start=False, stop=True)
                    h2ch, c2 = gate_math(g2, bias["b2c"], c2, False, ("2", n_steps - 1))
                else:
                    mm_group(g2, w["a2w"], h1T, KH, 0, start=True, stop=True)
                    h2ch, c2 = gate_math(g2, bias["b2c"], None, True, ("2", 0))

            # state outputs
            nc.sync.dma_start(h1_out[:], h1ch[:])
            nc.sync.dma_start(c1_out[:], c1[:])
            nc.sync.dma_start(h2_out[:], h2ch[:])
            nc.sync.dma_start(c2_out[:], c2[:])

            # ================= head =================
            with nc.named_scope("head"):
                # mid chunk [512,B] = W1c.T @ [h1T; topicT] + bh1  -> f16
                midp = psum_banks("g1")
                mm_group(midp, w["w1w"], h1T, KH, 0, start=True, stop=False)
                mm_group(midp, w["w1w"], topict, KH, KH * GC, start=False, stop=True)
                mid16 = gm.tile([128, 4 * B], F16, tag="mid16")
                for m in range(4):
                    nc.vector.tensor_scalar_add(
                        mid16[:, m * B:(m + 1) * B], midp[m][:], bias["bh1"][:, m:m + 1])

                # partial outT [1024,B] f32 over this core's mid chunk
                pof32 = gm.tile([128, 8 * B], F32, tag="pof32")
                for m2 in range(8):
                    pb = ps.tile([128, B], F32, name=f"po_{m2}",
                                 tag=f"g{1 + m2 // 4}{m2 % 4}")
                    for k in range(4):
                        nc.tensor.matmul(
                            pb[:],
                            w["w2w"][:, k * OUTp + m2 * 128: k * OUTp + (m2 + 1) * 128],
                            mid16[:, k * B:(k + 1) * B],
                            start=(k == 0), stop=(k == 3))
                    nc.vector.tensor_copy(pof32[:, m2 * B:(m2 + 1) * B], pb[:])

                rsin = dr.tile([OUTp, B], F32, tag="rsin")
                nc.sync.dma_start(rsin.rearrange("(k p) n -> p k n", p=128),
                                  pof32.rearrange("p (k n) -> p k n", n=B))
                rsout = dr.tile([128, B], F32, tag="rsout")
                nc.gpsimd.collective_compute(
                    "ReduceScatter", ALU.add,
                    replica_groups=[list(range(N_CORES))],
                    ins=[rsin.opt()], outs=[rsout.opt()])

                mb = gm.tile([128, B], F32, tag="mb")
                nc.sync.dma_start(mb[:], rsout[:])
                m2b = gm.tile([128, B], F32, tag="m2b")
                nc.vector.tensor_scalar_add(m2b[:], mb[:], bias["bh2"][:, 0:1])
                negmax = gm.tile([128, 1], F32, tag="negmax")
                nc.vector.reduce_max(negmax[:], m2b[:], axis=mybir.AxisListType.X, negate=True)
                ex = gm.tile([128, B], F32, tag="ex")
                ssum = gm.tile([128, 1], F32, tag="ssum")
                nc.scalar.activation(ex[:], m2b[:], AF.Exp, bias=negmax[:], accum_out=ssum[:])
                rec = gm.tile([128, 1], F32, tag="rec")
                nc.vector.reciprocal(rec[:], ssum[:])
                smt = gm.tile([128, B], F32, tag="smt")
                nc.vector.tensor_scalar_mul(smt[:], ex[:], rec[:])
                nc.sync.dma_start(sm_out[:], smt[:])

    nc.compile()
    return nc


def _sbuf_layout(lhsT):
    """[K, M] -> [128, (K/128)*M] host layout (k-tiles side by side)."""
    K, M = lhsT.shape
    assert K % 128 == 0
    return np.ascontiguousarray(
        lhsT.reshape(K // 128, 128, M).transpose(1, 0, 2).reshape(128, (K // 128) * M))


def _prep_inputs(inputs):
    f32 = np.float32
    W_ih1 = np.asarray(inputs["W_ih1"], f32); W_hh1 = np.asarray(inputs["W_hh1"], f32)
    W_ih2 = np.asarray(inputs["W_ih2"], f32); W_hh2 = np.asarray(inputs["W_hh2"], f32)
    W1 = np.asarray(inputs["W1"], f32); W2 = np.asarray(inputs["W2"], f32)
    b1 = np.asarray(inputs["b_ih1"], f32) + np.asarray(inputs["b_hh1"], f32)
    b2 = np.asarray(inputs["b_ih2"], f32) + np.asarray(inputs["b_hh2"], f32)
    bw1 = np.asarray(inputs["b1"], f32); bw2 = np.asarray(inputs["b2"], f32)
    X = np.asarray(inputs["input"], f32)          # (B, T, D)
    hidx = np.asarray(inputs["h"])                # (B, 5) int

    def pad_gate(Wm, kin, kin_p):
        """(4000, kin) -> (4096, kin_p) in padded gate space."""
        out = np.zeros((4, Hp, kin_p), f32)
        out[:, :H, :kin] = Wm.reshape(4, H, kin)
        return out.reshape(4 * Hp, kin_p)

    A1 = pad_gate(W_ih1[:, :D], D, Dp)
    B1 = pad_gate(W_ih1[:, D:] + W_hh1, H, Hp)
    A2 = pad_gate(W_ih2[:, D:D + H], H, Hp)
    B2 = pad_gate(W_ih2[:, D + H:] + W_hh2, H, Hp)

    # W1 in per-core-chunk padded mid space; columns [h1 | topic] each padded
    W1p = np.zeros((MIDp, 2 * Hp), f32)
    for k in range(N_CORES):
        W1p[k * 512:k * 512 + 500, :H] = W1[k * 500:(k + 1) * 500, :H]
        W1p[k * 512:k * 512 + 500, Hp:Hp + H] = W1[k * 500:(k + 1) * 500, H:]
    W2p = np.zeros((OUTp, MIDp), f32)
    for k in range(N_CORES):
        W2p[:OUT, k * 512:k * 512 + 500] = W2[:, k * 500:(k + 1) * 500]

    b1p = np.zeros((4, Hp), f32); b1p[:, :H] = b1.reshape(4, H)
    b2p = np.zeros((4, Hp), f32); b2p[:, :H] = b2.reshape(4, H)
    bw1p = np.zeros(MIDp, f32)
    for k in range(N_CORES):
        bw1p[k * 512:k * 512 + 500] = bw1[k * 500:(k + 1) * 500]
    bw2p = np.zeros(OUTp, f32); bw2p[:OUT] = bw2

    # x in device layout: (T, 128, KX*B) f16, replicated
    Xt = np.zeros((T, Dp, B), f32)
    Xt[:, :D, :] = X.transpose(1, 2, 0)
    xdev = np.ascontiguousarray(
        Xt.reshape(T, KX, 128, B).transpose(0, 2, 1, 3).reshape(T, 128, KX * B)
    ).astype(np.float16)

    topic = np.zeros((Hp, B), f32)
    topic[hidx.T % Hp, np.arange(B)[None, :]] = 1.0
    tp = topic.copy(); tp[H:] = 0.0
    topict = _sbuf_layout(tp).astype(np.float16)

    in_maps = []
    for k in range(N_CORES):
        rows = np.concatenate([np.arange(g * Hp + 128 * k, g * Hp + 128 * (k + 1))
                               for g in range(4)])
        d = {
            "xdev": xdev,
            "topict": topict,
            "a1w": _sbuf_layout(np.ascontiguousarray(A1[rows].T)).astype(np.float16),
            "b1w": _sbuf_layout(np.ascontiguousarray(B1[rows].T)).astype(np.float16),
            "a2w": _sbuf_layout(np.ascontiguousarray(A2[rows].T)).astype(np.float16),
            "b2w": _sbuf_layout(np.ascontiguousarray(B2[rows].T)).astype(np.float16),
            "w1w": _sbuf_layout(np.ascontiguousarray(W1p[k * 512:(k + 1) * 512].T)).astype(np.float16),
            "w2w": _sbuf_layout(np.ascontiguousarray(W2p[:, k * 512:(k + 1) * 512].T)).astype(np.float16),
            "b1c": np.ascontiguousarray(b1p[:, 128 * k:128 * (k + 1)].T),
            "b2c": np.ascontiguousarray(b2p[:, 128 * k:128 * (k + 1)].T),
            "bh1": np.ascontiguousarray(bw1p[k * 512:(k + 1) * 512].reshape(4, 128).T),
            "bh2": np.ascontiguousarray(bw2p[128 * k:128 * (k + 1)].reshape(128, 1)),
        }
        in_maps.append(d)
    return in_maps


def kernel(trace=False, tmpdir=None, **inputs):
    if "nc" not in _CACHE:
        _CACHE["nc"] = _build_nc()
    nc = _CACHE["nc"]
    in_maps = _prep_inputs(inputs)
    kw = {}
    if trace:
        kw = dict(trace=True, tmpdir=tmpdir)
    r = bass_utils.run_bass_kernel_spmd(nc, in_maps, core_ids=list(range(N_CORES)), **kw)
    _CACHE["last_results"] = r
    res = r.results

    sm = np.concatenate([res[k]["sm"] for k in range(N_CORES)], axis=0)
    h1 = np.concatenate([res[k]["h1c"] for k in range(N_CORES)], axis=0).astype(np.float32)
    c1 = np.concatenate([res[k]["c1c"] for k in range(N_CORES)], axis=0)
    h2 = np.concatenate([res[k]["h2c"] for k in range(N_CORES)], axis=0).astype(np.float32)
    c2 = np.concatenate([res[k]["c2c"] for k in range(N_CORES)], axis=0)

    softmax = np.ascontiguousarray(sm[:OUT].T)
    return (softmax,
            (np.ascontiguousarray(h1[:H].T), np.ascontiguousarray(c1[:H].T)),
            (np.ascontiguousarray(h2[:H].T), np.ascontiguousarray(c2[:H].T)))


# revision 6
# speedup vs baseline: 1.0118x; 1.0038x over previous
"""Trainium2 Bass kernel for nn_CueWordSelectNet (2-layer LSTM + MLP + batch-softmax).

Strategy (8 NeuronCores, SPMD):
  - Hidden/gate dimension sharded 8 ways: core k owns hidden rows [128k, 128k+128)
    of both LSTMs (padded 1000->1024). Weights are replicated-sliced per core and
    stay resident in SBUF across all 64 timesteps.
  - Everything lives in "hT layout": [hidden -> partitions, batch -> free dim],
    so the recurrent matmuls are out[gate,batch] = W[K=hid,M=gate].T @ hT[K,N=batch]
    with N=256 (full batch) and no transposes anywhere.
  - Algebraic fusion: gates1 = x_t@A1.T + h1@B1.T with B1 = W_ih1[:,600:]+W_hh1;
    gates2 = h1@A2.T + h2@B2.T with A2 = W_ih2[:,600:1600], B2 = W_ih2[:,1600:]+W_hh2
    (the zero block of encoder2's input is dropped). This cuts FLOPs ~35%.
  - Per step, each core computes its 512 gate rows (4 gates x 128 hidden), applies
    the LSTM cell math (fp32 PSUM + fp32 c state, fp16 h), and the 8 h-chunks are
    recombined with an AllGather. The loop is restructured so iteration t computes
    h2(t) then h1(t+1): two AllGathers per iteration that pipeline with compute.
  - Head: mid = [h1;topic]@W1.T+b1 sharded by mid rows, then out partials
    W2-chunk.T@mid-chunk ReduceScattered (add) so each core lands exactly its
    output rows; softmax over the batch (free) dim is local per core.
  - Matmul dtype fp16 (1 cycle/row on PE, ~10x tighter than bf16; fp32 accumulate).

Host side only shards/pads/transposes inputs and reassembles outputs.
"""
import numpy as np

import concourse.bacc as bacc
import concourse.mybir as mybir
import concourse.tile as tile
from concourse import bass_utils

F16 = mybir.dt.float16
F32 = mybir.dt.float32
AF = mybir.ActivationFunctionType
ALU = mybir.AluOpType

N_CORES = 8
B = 256          # batch (free dim everywhere)
T = 64
D, Dp = 600, 640         # input size, padded (5 k-tiles)
H, Hp = 1000, 1024       # hidden, padded (8 k-tiles)
GC = 512                 # per-core gate rows (4 gates x 128)
MID, MIDp = 4000, 4096   # head inner dim, padded (512 per core)
OUT, OUTp = 1000, 1024   # head output dim, padded (128 per core)

KH = Hp // 128   # 8 hidden k-tiles
KX = Dp // 128   # 5 x k-tiles

_CACHE = {}


def _build_nc(n_steps=T):
    nc = bacc.Bacc("TRN2", target_bir_lowering=False, debug=False, num_devices=N_CORES)

    # ---- per-core external inputs (all pre-laid-out on host) ----
    din = {}
    din["xdev"] = nc.dram_tensor("xdev", [n_steps, 128, KX * B], F16, kind="ExternalInput").ap()
    din["topict"] = nc.dram_tensor("topict", [128, KH * B], F16, kind="ExternalInput").ap()
    for name, cols in [("a1w", KX * GC), ("b1w", KH * GC), ("a2w", KH * GC),
                       ("b2w", KH * GC), ("w1w", 2 * KH * GC), ("w2w", 4 * OUTp)]:
        din[name] = nc.dram_tensor(name, [128, cols], F16, kind="ExternalInput").ap()
    for name, cols in [("b1c", 4), ("b2c", 4), ("bh1", 4), ("bh2", 1)]:
        din[name] = nc.dram_tensor(name, [128, cols], F32, kind="ExternalInput").ap()

    # ---- per-core external outputs ----
    sm_out = nc.dram_tensor("sm", [128, B], F32, kind="ExternalOutput").ap()
    h1_out = nc.dram_tensor("h1c", [128, B], F16, kind="ExternalOutput").ap()
    c1_out = nc.dram_tensor("c1c", [128, B], F32, kind="ExternalOutput").ap()
    h2_out = nc.dram_tensor("h2c", [128, B], F16, kind="ExternalOutput").ap()
    c2_out = nc.dram_tensor("c2c", [128, B], F32, kind="ExternalOutput").ap()

    with tile.TileContext(nc) as tc:
        with (
            tc.tile_pool(name="wp", bufs=1) as wp,          # persistent weights
            tc.tile_pool(name="hp", bufs=2) as hp,          # gathered h states
            tc.tile_pool(name="xp", bufs=3) as xp,          # x_t prefetch
            tc.tile_pool(name="gm", bufs=3) as gm,          # gate-math temporaries
            tc.tile_pool(name="cs", bufs=2) as cs,          # c states
            tc.tile_pool(name="ps", bufs=1, space="PSUM") as ps,
            tc.tile_pool(name="dr", bufs=2, space="DRAM") as dr,
        ):
            # ---- load resident weights/biases ----
            w = {}
            for name in ("a1w", "b1w", "a2w", "b2w", "w1w", "w2w"):
                cols = din[name].shape[-1]
                w[name] = wp.tile([128, cols], F16, name=f"sb_{name}", tag=name)
                nc.sync.dma_start(w[name][:], din[name][:])
            bias = {}
            for name in ("b1c", "b2c", "bh1", "bh2"):
                cols = din[name].shape[-1]
                bias[name] = wp.tile([128, cols], F32, name=f"sb_{name}", tag=name)
                nc.sync.dma_start(bias[name][:], din[name][:])
            topict = wp.tile([128, KH * B], F16, tag="topict")
            nc.sync.dma_start(topict[:], din["topict"][:])

            # psum banks: 4 for gates1, 4 for gates2 (reused by the head)
            def psum_banks(prefix):
                return [ps.tile([128, B], F32, name=f"{prefix}{m}", tag=f"{prefix}{m}")
                        for m in range(4)]

            def load_x(t):
                xt = xp.tile([128, KX * B], F16, name=f"x_{t}", tag="xt")
                nc.sync.dma_start(xt[:], din["xdev"][t])
                return xt

            def gate_math(pb, bias_ap, c_prev, first, idx):
                """LSTM cell math from 4 psum banks; returns (h_chunk f16, c_new f32)."""
                si = gm.tile([128, B], F32, name=f"si_{idx}", tag="si")
                tg = gm.tile([128, B], F32, name=f"tg_{idx}", tag="tg")
                so = gm.tile([128, B], F32, name=f"so_{idx}", tag="so")
                nc.scalar.activation(si[:], pb[0][:], AF.Sigmoid, bias=bias_ap[:, 0:1])
                nc.scalar.activation(tg[:], pb[2][:], AF.Tanh, bias=bias_ap[:, 2:3])
                nc.scalar.activation(so[:], pb[3][:], AF.Sigmoid, bias=bias_ap[:, 3:4])
                cn = cs.tile([128, B], F32, name=f"c_{idx}", tag=f"c{idx[0]}")
                if first:
                    nc.vector.tensor_tensor(cn[:], si[:], tg[:], ALU.mult)
                else:
                    sf = gm.tile([128, B], F32, name=f"sf_{idx}", tag="sf")
                    nc.scalar.activation(sf[:], pb[1][:], AF.Sigmoid, bias=bias_ap[:, 1:2])
                    t1 = gm.tile([128, B], F32, name=f"t1_{idx}", tag="t1")
                    nc.vector.tensor_tensor(t1[:], sf[:], c_prev[:], ALU.mult)
                    t2 = gm.tile([128, B], F32, name=f"t2_{idx}", tag="t2")
                    nc.vector.tensor_tensor(t2[:], si[:], tg[:], ALU.mult)
                    nc.vector.tensor_tensor(cn[:], t1[:], t2[:], ALU.add)
                tcn = gm.tile([128, B], F32, name=f"tcn_{idx}", tag="tcn")
                nc.scalar.activation(tcn[:], cn[:], AF.Tanh)
                hch = gm.tile([128, B], F16, name=f"h_{idx}", tag=f"hch{idx[0]}")
                nc.vector.tensor_tensor(hch[:], so[:], tcn[:], ALU.mult)
                return hch, cn

            def gather(hch, idx):
                """AllGather h chunk -> full hT [128, KH*B] in SBUF."""
                bin_ = dr.tile([128, B], F16, name=f"bin_{idx}", tag=f"bin{idx[0]}")
                bout = dr.tile([Hp, B], F16, name=f"bout_{idx}", tag=f"bout{idx[0]}")
                nc.sync.dma_start(bin_[:], hch[:])
                nc.gpsimd.collective_compute(
                    "AllGather", ALU.bypass,
                    replica_groups=[list(range(N_CORES))],
                    ins=[bin_.opt()], outs=[bout.opt()])
                hT = hp.tile([128, KH * B], F16, name=f"hT_{idx}", tag=f"hT{idx[0]}")
                # per-k-tile DMAs so the first k-tile's matmuls start early
                for k in range(KH):
                    nc.sync.dma_start(hT[:, k * B:(k + 1) * B],
                                      bout[128 * k:128 * (k + 1), :])
                return hT

            def mm_group(pb, wt, rhs, ktiles, woff, start, stop):
                """Accumulate ktiles matmuls into the 4 psum banks."""
                for m in range(4):
                    for k in range(ktiles):
                        nc.tensor.matmul(
                            pb[m][:],
                            wt[:, woff + k * GC + m * 128: woff + k * GC + (m + 1) * 128],
                            rhs[:, k * B:(k + 1) * B],
                            start=(start and k == 0),
                            stop=(stop and k == ktiles - 1))

            # ================= prologue: t = 0 =================
            x0 = load_x(0)
            g1 = psum_banks("g1")
            mm_group(g1, w["a1w"], x0, KX, 0, start=True, stop=True)
            h1ch, c1 = gate_math(g1, bias["b1c"], None, True, ("1", 0))
            h1T = gather(h1ch, ("1", 0))

            h2T, c2, h2ch = None, None, None
            x_next = load_x(1) if n_steps > 1 else None

            # ================= main loop =================
            # iteration t: computes h1(t+1) chunk first (critical AG path),
            # then h2(t); A2/B2 parts ride in the AG stall windows.
            for t in range(n_steps - 1):
                with nc.named_scope(f"step{t}"):
                    first2 = (t == 0)
                    # g1(t+1) first: x+B1 inputs landed early (AG1(t-1) ran
                    # before AG2(t-1)), so gm1 finishes during AG2(t-1) and
                    # AG1(t) triggers as soon as the cc stream frees up.
                    g1 = psum_banks("g1")
                    mm_group(g1, w["a1w"], x_next, KX, 0, start=True, stop=False)
                    mm_group(g1, w["b1w"], h1T, KH, 0, start=False, stop=True)
                    h1ch, c1 = gate_math(g1, bias["b1c"], c1, False, ("1", t + 1))
                    h1T_new = gather(h1ch, ("1", t + 1))
                    # g2(t) rides in AG1(t)'s flight window
                    g2 = psum_banks("g2")
                    if not first2:
                        mm_group(g2, w["b2w"], h2T, KH, 0, start=True, stop=False)
                    mm_group(g2, w["a2w"], h1T, KH, 0, start=first2, stop=True)
                    h2ch, c2 = gate_math(g2, bias["b2c"], c2, first2, ("2", t))
                    h2T = gather(h2ch, ("2", t))
                    if t + 2 < n_steps:
                        x_next = load_x(t + 2)
                    h1T = h1T_new

            # ================= epilogue: gates2(T-1) =================
            with nc.named_scope("epilogue"):
                g2 = psum_banks("g2")
                if n_steps > 1:
                    mm_group(g2, w["b2w"], h2T, KH, 0, start=True, stop=False)
                    mm_group(g2, w["a2w"], h1T, KH, 0, start=False, stop=True)
                    h2ch, c2 = gate_math(g2, bias["b2c"], c2, False, ("2", n_steps - 1))
                else:
                    mm_group(g2, w["a2w"], h1T, KH, 0, start=True, stop=True)
                    h2ch, c2 = gate_math(g2, bias["b2c"], None, True, ("2", 0))

            # state outputs
            nc.sync.dma_start(h1_out[:], h1ch[:])
            nc.sync.dma_start(c1_out[:], c1[:])
            nc.sync.dma_start(h2_out[:], h2ch[:])
            nc.sync.dma_start(c2_out[:], c2[:])

            # ================= head =================
            with nc.named_scope("head"):
                # mid chunk [512,B] = W1c.T @ [h1T; topicT] + bh1  -> f16
                midp = psum_banks("g1")
                mm_group(midp, w["w1w"], h1T, KH, 0, start=True, stop=False)
                mm_group(midp, w["w1w"], topict, KH, KH * GC, start=False, stop=True)
                mid16 = gm.tile([128, 4 * B], F16, tag="mid16")
                for m in range(4):
                    nc.vector.tensor_scalar_add(
                        mid16[:, m * B:(m + 1) * B], midp[m][:], bias["bh1"][:, m:m + 1])

                # partial outT [1024,B] f32 over this core's mid chunk
                pof32 = gm.tile([128, 8 * B], F32, tag="pof32")
                for m2 in range(8):
                    pb = ps.tile([128, B], F32, name=f"po_{m2}",
                                 tag=f"g{1 + m2 // 4}{m2 % 4}")
                    for k in range(4):
                        nc.tensor.matmul(
                            pb[:],
                            w["w2w"][:, k * OUTp + m2 * 128: k * OUTp + (m2 + 1) * 128],
                            mid16[:, k * B:(k + 1) * B],
                            start=(k == 0), stop=(k == 3))
                    nc.vector.tensor_copy(pof32[:, m2 * B:(m2 + 1) * B], pb[:])

                rsin = dr.tile([OUTp, B], F32, tag="rsin")
                nc.sync.dma_start(rsin.rearrange("(k p) n -> p k n", p=128),
                                  pof32.rearrange("p (k n) -> p k n", n=B))
                rsout = dr.tile([128, B], F32, tag="rsout")
                nc.gpsimd.collective_compute(
                    "ReduceScatter", ALU.add,
                    replica_groups=[list(range(N_CORES))],
                    ins=[rsin.opt()], outs=[rsout.opt()])

                mb = gm.tile([128, B], F32, tag="mb")
                nc.sync.dma_start(mb[:], rsout[:])
                m2b = gm.tile([128, B], F32, tag="m2b")
                nc.vector.tensor_scalar_add(m2b[:], mb[:], bias["bh2"][:, 0:1])
                negmax = gm.tile([128, 1], F32, tag="negmax")
                nc.vector.reduce_max(negmax[:], m2b[:], axis=mybir.AxisListType.X, negate=True)
                ex = gm.tile([128, B], F32, tag="ex")
                ssum = gm.tile([128, 1], F32, tag="ssum")
                nc.scalar.activation(ex[:], m2b[:], AF.Exp, bias=negmax[:], accum_out=ssum[:])
                rec = gm.tile([128, 1], F32, tag="rec")
                nc.vector.reciprocal(rec[:], ssum[:])
                smt = gm.tile([128, B], F32, tag="smt")
                nc.vector.tensor_scalar_mul(smt[:], ex[:], rec[:])
                nc.sync.dma_start(sm_out[:], smt[:])

    nc.compile()
    return nc


def _sbuf_layout(lhsT):
    """[K, M] -> [128, (K/128)*M] host layout (k-tiles side by side)."""
    K, M = lhsT.shape
    assert K % 128 == 0
    return np.ascontiguousarray(
        lhsT.reshape(K // 128, 128, M).transpose(1, 0, 2).reshape(128, (K // 128) * M))


def _prep_inputs(inputs):
    f32 = np.float32
    W_ih1 = np.asarray(inputs["W_ih1"], f32); W_hh1 = np.asarray(inputs["W_hh1"], f32)
    W_ih2 = np.asarray(inputs["W_ih2"], f32); W_hh2 = np.asarray(inputs["W_hh2"], f32)
    W1 = np.asarray(inputs["W1"], f32); W2 = np.asarray(inputs["W2"], f32)
    b1 = np.asarray(inputs["b_ih1"], f32) + np.asarray(inputs["b_hh1"], f32)
    b2 = np.asarray(inputs["b_ih2"], f32) + np.asarray(inputs["b_hh2"], f32)
    bw1 = np.asarray(inputs["b1"], f32); bw2 = np.asarray(inputs["b2"], f32)
    X = np.asarray(inputs["input"], f32)          # (B, T, D)
    hidx = np.asarray(inputs["h"])                # (B, 5) int

    def pad_gate(Wm, kin, kin_p):
        """(4000, kin) -> (4096, kin_p) in padded gate space."""
        out = np.zeros((4, Hp, kin_p), f32)
        out[:, :H, :kin] = Wm.reshape(4, H, kin)
        return out.reshape(4 * Hp, kin_p)

    A1 = pad_gate(W_ih1[:, :D], D, Dp)
    B1 = pad_gate(W_ih1[:, D:] + W_hh1, H, Hp)
    A2 = pad_gate(W_ih2[:, D:D + H], H, Hp)
    B2 = pad_gate(W_ih2[:, D + H:] + W_hh2, H, Hp)

    # W1 in per-core-chunk padded mid space; columns [h1 | topic] each padded
    W1p = np.zeros((MIDp, 2 * Hp), f32)
    for k in range(N_CORES):
        W1p[k * 512:k * 512 + 500, :H] = W1[k * 500:(k + 1) * 500, :H]
        W1p[k * 512:k * 512 + 500, Hp:Hp + H] = W1[k * 500:(k + 1) * 500, H:]
    W2p = np.zeros((OUTp, MIDp), f32)
    for k in range(N_CORES):
        W2p[:OUT, k * 512:k * 512 + 500] = W2[:, k * 500:(k + 1) * 500]

    b1p = np.zeros((4, Hp), f32); b1p[:, :H] = b1.reshape(4, H)
    b2p = np.zeros((4, Hp), f32); b2p[:, :H] = b2.reshape(4, H)
    bw1p = np.zeros(MIDp, f32)
    for k in range(N_CORES):
        bw1p[k * 512:k * 512 + 500] = bw1[k * 500:(k + 1) * 500]
    bw2p = np.zeros(OUTp, f32); bw2p[:OUT] = bw2

    # x in device layout: (T, 128, KX*B) f16, replicated
    Xt = np.zeros((T, Dp, B), f32)
    Xt[:, :D, :] = X.transpose(1, 2, 0)
    xdev = np.ascontiguousarray(
        Xt.reshape(T, KX, 128, B).transpose(0, 2, 1, 3).reshape(T, 128, KX * B)
    ).astype(np.float16)

    topic = np.zeros((Hp, B), f32)
    topic[hidx.T % Hp, np.arange(B)[None, :]] = 1.0
    tp = topic.copy(); tp[H:] = 0.0
    topict = _sbuf_layout(tp).astype(np.float16)

    in_maps = []
    for k in range(N_CORES):
        rows = np.concatenate([np.arange(g * Hp + 128 * k, g * Hp + 128 * (k + 1))
                               for g in range(4)])
        d = {
            "xdev": xdev,
            "topict": topict,
            "a1w": _sbuf_layout(np.ascontiguousarray(A1[rows].T)).astype(np.float16),
            "b1w": _sbuf_layout(np.ascontiguousarray(B1[rows].T)).astype(np.float16),
            "a2w": _sbuf_layout(np.ascontiguousarray(A2[rows].T)).astype(np.float16),
            "b2w": _sbuf_layout(np.ascontiguousarray(B2[rows].T)).astype(np.float16),
            "w1w": _sbuf_layout(np.ascontiguousarray(W1p[k * 512:(k + 1) * 512].T)).astype(np.float16),
            "w2w": _sbuf_layout(np.ascontiguousarray(W2p[:, k * 512:(k + 1) * 512].T)).astype(np.float16),
            "b1c": np.ascontiguousarray(b1p[:, 128 * k:128 * (k + 1)].T),
            "b2c": np.ascontiguousarray(b2p[:, 128 * k:128 * (k + 1)].T),
            "bh1": np.ascontiguousarray(bw1p[k * 512:(k + 1) * 512].reshape(4, 128).T),
            "bh2": np.ascontiguousarray(bw2p[128 * k:128 * (k + 1)].reshape(128, 1)),
        }
        in_maps.append(d)
    return in_maps


def kernel(trace=False, tmpdir=None, **inputs):
    if "nc" not in _CACHE:
        _CACHE["nc"] = _build_nc()
    nc = _CACHE["nc"]
    in_maps = _prep_inputs(inputs)
    kw = {}
    if trace:
        kw = dict(trace=True, tmpdir=tmpdir)
    r = bass_utils.run_bass_kernel_spmd(nc, in_maps, core_ids=list(range(N_CORES)), **kw)
    _CACHE["last_results"] = r
    res = r.results

    sm = np.concatenate([res[k]["sm"] for k in range(N_CORES)], axis=0)
    h1 = np.concatenate([res[k]["h1c"] for k in range(N_CORES)], axis=0).astype(np.float32)
    c1 = np.concatenate([res[k]["c1c"] for k in range(N_CORES)], axis=0)
    h2 = np.concatenate([res[k]["h2c"] for k in range(N_CORES)], axis=0).astype(np.float32)
    c2 = np.concatenate([res[k]["c2c"] for k in range(N_CORES)], axis=0)

    softmax = np.ascontiguousarray(sm[:OUT].T)
    return (softmax,
            (np.ascontiguousarray(h1[:H].T), np.ascontiguousarray(c1[:H].T)),
            (np.ascontiguousarray(h2[:H].T), np.ascontiguousarray(c2[:H].T)))


# revision 9
# speedup vs baseline: 1.2557x; 1.2411x over previous
"""Trainium2 Bass kernel for nn_CueWordSelectNet (2-layer LSTM + MLP + batch-softmax).

Strategy (8 NeuronCores, SPMD):
  - Hidden/gate dimension sharded 8 ways: core k owns hidden rows [128k, 128k+128)
    of both LSTMs (padded 1000->1024). Weights are replicated-sliced per core and
    stay resident in SBUF across all 64 timesteps.
  - Everything lives in "hT layout": [hidden -> partitions, batch -> free dim],
    so the recurrent matmuls are out[gate,batch] = W[K=hid,M=gate].T @ hT[K,N=batch]
    with N=256 (full batch) and no transposes anywhere.
  - Algebraic fusion: gates1 = x_t@A1.T + h1@B1.T with B1 = W_ih1[:,600:]+W_hh1;
    gates2 = h1@A2.T + h2@B2.T with A2 = W_ih2[:,600:1600], B2 = W_ih2[:,1600:]+W_hh2
    (the zero block of encoder2's input is dropped). This cuts FLOPs ~35%.
  - Per step, each core computes its 512 gate rows (4 gates x 128 hidden), applies
    the LSTM cell math (fp32 PSUM + fp32 c state, fp16 h), and the 8 h-chunks are
    recombined with an AllGather. The loop is restructured so iteration t computes
    h2(t) then h1(t+1): two AllGathers per iteration that pipeline with compute.
  - Head: mid = [h1;topic]@W1.T+b1 sharded by mid rows, then out partials
    W2-chunk.T@mid-chunk ReduceScattered (add) so each core lands exactly its
    output rows; softmax over the batch (free) dim is local per core.
  - Matmul dtype fp16 (1 cycle/row on PE, ~10x tighter than bf16; fp32 accumulate).

Host side only shards/pads/transposes inputs and reassembles outputs.
"""
import numpy as np

import concourse.bacc as bacc
import concourse.mybir as mybir
import concourse.tile as tile
from concourse import bass_utils

F16 = mybir.dt.float16
F32 = mybir.dt.float32
AF = mybir.ActivationFunctionType
ALU = mybir.AluOpType

N_CORES = 8
B = 256          # batch (free dim everywhere)
T = 64
D, Dp = 600, 640         # input size, padded (5 k-tiles)
H, Hp = 1000, 1024       # hidden, padded (8 k-tiles)
GC = 512                 # per-core gate rows (4 gates x 128)
MID, MIDp = 4000, 4096   # head inner dim, padded (512 per core)
OUT, OUTp = 1000, 1024   # head output dim, padded (128 per core)

KH = Hp // 128   # 8 hidden k-tiles
KX = Dp // 128   # 5 x k-tiles

_CACHE = {}


def _build_nc(n_steps=T):
    nc = bacc.Bacc("TRN2", target_bir_lowering=False, debug=False, num_devices=N_CORES)

    # ---- per-core external inputs (all pre-laid-out on host) ----
    din = {}
    din["xdev"] = nc.dram_tensor("xdev", [n_steps, 128, KX * B], F16, kind="ExternalInput").ap()
    din["topict"] = nc.dram_tensor("topict", [128, KH * B], F16, kind="ExternalInput").ap()
    for name, cols in [("a1w", KX * GC), ("b1w", KH * GC), ("a2w", KH * GC),
                       ("b2w", KH * GC), ("w1w", 2 * KH * GC), ("w2w", 4 * OUTp)]:
        din[name] = nc.dram_tensor(name, [128, cols], F16, kind="ExternalInput").ap()
    for name, cols in [("b1c", 4), ("b2c", 4), ("bh1", 4), ("bh2", 1)]:
        din[name] = nc.dram_tensor(name, [128, cols], F32, kind="ExternalInput").ap()

    # ---- per-core external outputs ----
    sm_out = nc.dram_tensor("sm", [128, B], F32, kind="ExternalOutput").ap()
    h1_out = nc.dram_tensor("h1c", [128, B], F16, kind="ExternalOutput").ap()
    c1_out = nc.dram_tensor("c1c", [128, B], F32, kind="ExternalOutput").ap()
    h2_out = nc.dram_tensor("h2c", [128, B], F16, kind="ExternalOutput").ap()
    c2_out = nc.dram_tensor("c2c", [128, B], F32, kind="ExternalOutput").ap()

    with tile.TileContext(nc) as tc:
        with (
            tc.tile_pool(name="wp", bufs=1) as wp,          # persistent weights
            tc.tile_pool(name="hp", bufs=2) as hp,          # gathered h states
            tc.tile_pool(name="xp", bufs=3) as xp,          # x_t prefetch
            tc.tile_pool(name="gm", bufs=3) as gm,          # gate-math temporaries
            tc.tile_pool(name="cs", bufs=2) as cs,          # c states
            tc.tile_pool(name="ps", bufs=1, space="PSUM") as ps,
            tc.tile_pool(name="dr", bufs=2, space="DRAM") as dr,
        ):
            # ---- load resident weights/biases ----
            w = {}
            for name in ("a1w", "b1w", "a2w", "b2w", "w1w", "w2w"):
                cols = din[name].shape[-1]
                w[name] = wp.tile([128, cols], F16, name=f"sb_{name}", tag=name)
                nc.sync.dma_start(w[name][:], din[name][:])
            bias = {}
            for name in ("b1c", "b2c", "bh1", "bh2"):
                cols = din[name].shape[-1]
                bias[name] = wp.tile([128, cols], F32, name=f"sb_{name}", tag=name)
                nc.sync.dma_start(bias[name][:], din[name][:])
            topict = wp.tile([128, KH * B], F16, tag="topict")
            nc.sync.dma_start(topict[:], din["topict"][:])

            # psum banks: 4 for gates1, 4 for gates2 (reused by the head)
            def psum_banks(prefix):
                return [ps.tile([128, B], F32, name=f"{prefix}{m}", tag=f"{prefix}{m}")
                        for m in range(4)]

            def load_x(t):
                xt = xp.tile([128, KX * B], F16, name=f"x_{t}", tag="xt")
                nc.gpsimd.dma_start(xt[:], din["xdev"][t])
                return xt

            def gate_math(pb, bias_ap, c_prev, first, idx):
                """LSTM cell math from 4 psum banks; returns (h_chunk f16, c_new f32)."""
                si = gm.tile([128, B], F32, name=f"si_{idx}", tag="si")
                tg = gm.tile([128, B], F32, name=f"tg_{idx}", tag="tg")
                so = gm.tile([128, B], F32, name=f"so_{idx}", tag="so")
                nc.scalar.activation(si[:], pb[0][:], AF.Sigmoid, bias=bias_ap[:, 0:1])
                nc.scalar.activation(tg[:], pb[2][:], AF.Tanh, bias=bias_ap[:, 2:3])
                nc.scalar.activation(so[:], pb[3][:], AF.Sigmoid, bias=bias_ap[:, 3:4])
                cn = cs.tile([128, B], F32, name=f"c_{idx}", tag=f"c{idx[0]}")
                if first:
                    nc.vector.tensor_tensor(cn[:], si[:], tg[:], ALU.mult)
                else:
                    sf = gm.tile([128, B], F32, name=f"sf_{idx}", tag="sf")
                    nc.scalar.activation(sf[:], pb[1][:], AF.Sigmoid, bias=bias_ap[:, 1:2])
                    t1 = gm.tile([128, B], F32, name=f"t1_{idx}", tag="t1")
                    nc.vector.tensor_tensor(t1[:], sf[:], c_prev[:], ALU.mult)
                    t2 = gm.tile([128, B], F32, name=f"t2_{idx}", tag="t2")
                    nc.vector.tensor_tensor(t2[:], si[:], tg[:], ALU.mult)
                    nc.vector.tensor_tensor(cn[:], t1[:], t2[:], ALU.add)
                tcn = gm.tile([128, B], F32, name=f"tcn_{idx}", tag="tcn")
                nc.scalar.activation(tcn[:], cn[:], AF.Tanh)
                hch = gm.tile([128, B], F16, name=f"h_{idx}", tag=f"hch{idx[0]}")
                nc.vector.tensor_tensor(hch[:], so[:], tcn[:], ALU.mult)
                return hch, cn

            def gather(hch, idx):
                """AllGather h chunk -> full hT [128, KH*B] in SBUF.

                The two AG paths use different engines' DMA queues (sync for
                LSTM1, scalar for LSTM2) so their transfers don't serialize."""
                eng = nc.sync if idx[0] == "1" else nc.scalar
                bin_ = dr.tile([128, B], F16, name=f"bin_{idx}", tag=f"bin{idx[0]}")
                bout = dr.tile([Hp, B], F16, name=f"bout_{idx}", tag=f"bout{idx[0]}")
                eng.dma_start(bin_[:], hch[:])
                nc.gpsimd.collective_compute(
                    "AllGather", ALU.bypass,
                    replica_groups=[list(range(N_CORES))],
                    ins=[bin_.opt()], outs=[bout.opt()])
                hT = hp.tile([128, KH * B], F16, name=f"hT_{idx}", tag=f"hT{idx[0]}")
                eng.dma_start(hT.rearrange("p (k n) -> p k n", n=B),
                              bout.rearrange("(k p) n -> p k n", p=128))
                return hT

            def mm_group(pb, wt, rhs, ktiles, woff, start, stop):
                """Accumulate ktiles matmuls into the 4 psum banks."""
                for m in range(4):
                    for k in range(ktiles):
                        nc.tensor.matmul(
                            pb[m][:],
                            wt[:, woff + k * GC + m * 128: woff + k * GC + (m + 1) * 128],
                            rhs[:, k * B:(k + 1) * B],
                            start=(start and k == 0),
                            stop=(stop and k == ktiles - 1))

            # ================= prologue: t = 0 =================
            x0 = load_x(0)
            g1 = psum_banks("g1")
            mm_group(g1, w["a1w"], x0, KX, 0, start=True, stop=True)
            h1ch, c1 = gate_math(g1, bias["b1c"], None, True, ("1", 0))
            h1T = gather(h1ch, ("1", 0))

            h2T, c2, h2ch = None, None, None
            x_next = load_x(1) if n_steps > 1 else None

            # ================= main loop =================
            # iteration t: computes h1(t+1) chunk first (critical AG path),
            # then h2(t); A2/B2 parts ride in the AG stall windows.
            for t in range(n_steps - 1):
                with nc.named_scope(f"step{t}"):
                    first2 = (t == 0)
                    # g1(t+1) first: x+B1 inputs landed early (AG1(t-1) ran
                    # before AG2(t-1)), so gm1 finishes during AG2(t-1) and
                    # AG1(t) triggers as soon as the cc stream frees up.
                    g1 = psum_banks("g1")
                    mm_group(g1, w["a1w"], x_next, KX, 0, start=True, stop=False)
                    mm_group(g1, w["b1w"], h1T, KH, 0, start=False, stop=True)
                    h1ch, c1 = gate_math(g1, bias["b1c"], c1, False, ("1", t + 1))
                    h1T_new = gather(h1ch, ("1", t + 1))
                    # g2(t) rides in AG1(t)'s flight window
                    g2 = psum_banks("g2")
                    if not first2:
                        mm_group(g2, w["b2w"], h2T, KH, 0, start=True, stop=False)
                    mm_group(g2, w["a2w"], h1T, KH, 0, start=first2, stop=True)
                    h2ch, c2 = gate_math(g2, bias["b2c"], c2, first2, ("2", t))
                    h2T = gather(h2ch, ("2", t))
                    if t + 2 < n_steps:
                        x_next = load_x(t + 2)
                    h1T = h1T_new

            # ================= epilogue: gates2(T-1) =================
            with nc.named_scope("epilogue"):
                g2 = psum_banks("g2")
                if n_steps > 1:
                    mm_group(g2, w["b2w"], h2T, KH, 0, start=True, stop=False)
                    mm_group(g2, w["a2w"], h1T, KH, 0, start=False, stop=True)
                    h2ch, c2 = gate_math(g2, bias["b2c"], c2, False, ("2", n_steps - 1))
                else:
                    mm_group(g2, w["a2w"], h1T, KH, 0, start=True, stop=True)
                    h2ch, c2 = gate_math(g2, bias["b2c"], None, True, ("2", 0))

            # state outputs
            nc.sync.dma_start(h1_out[:], h1ch[:])
            nc.sync.dma_start(c1_out[:], c1[:])
            nc.sync.dma_start(h2_out[:], h2ch[:])
            nc.sync.dma_start(c2_out[:], c2[:])

            # ================= head =================
            with nc.named_scope("head"):
                # mid chunk [512,B] = W1c.T @ [h1T; topicT] + bh1  -> f16
                midp = psum_banks("g1")
                mm_group(midp, w["w1w"], h1T, KH, 0, start=True, stop=False)
                mm_group(midp, w["w1w"], topict, KH, KH * GC, start=False, stop=True)
                mid16 = gm.tile([128, 4 * B], F16, tag="mid16")
                for m in range(4):
                    nc.vector.tensor_scalar_add(
                        mid16[:, m * B:(m + 1) * B], midp[m][:], bias["bh1"][:, m:m + 1])

                # partial outT [1024,B] f32 over this core's mid chunk
                pof32 = gm.tile([128, 8 * B], F32, tag="pof32")
                for m2 in range(8):
                    pb = ps.tile([128, B], F32, name=f"po_{m2}",
                                 tag=f"g{1 + m2 // 4}{m2 % 4}")
                    for k in range(4):
                        nc.tensor.matmul(
                            pb[:],
                            w["w2w"][:, k * OUTp + m2 * 128: k * OUTp + (m2 + 1) * 128],
                            mid16[:, k * B:(k + 1) * B],
                            start=(k == 0), stop=(k == 3))
                    nc.vector.tensor_copy(pof32[:, m2 * B:(m2 + 1) * B], pb[:])

                rsin = dr.tile([OUTp, B], F32, tag="rsin")
                nc.sync.dma_start(rsin.rearrange("(k p) n -> p k n", p=128),
                                  pof32.rearrange("p (k n) -> p k n", n=B))
                rsout = dr.tile([128, B], F32, tag="rsout")
                nc.gpsimd.collective_compute(
                    "ReduceScatter", ALU.add,
                    replica_groups=[list(range(N_CORES))],
                    ins=[rsin.opt()], outs=[rsout.opt()])

                mb = gm.tile([128, B], F32, tag="mb")
                nc.sync.dma_start(mb[:], rsout[:])
                m2b = gm.tile([128, B], F32, tag="m2b")
                nc.vector.tensor_scalar_add(m2b[:], mb[:], bias["bh2"][:, 0:1])
                negmax = gm.tile([128, 1], F32, tag="negmax")
                nc.vector.reduce_max(negmax[:], m2b[:], axis=mybir.AxisListType.X, negate=True)
                ex = gm.tile([128, B], F32, tag="ex")
                ssum = gm.tile([128, 1], F32, tag="ssum")
                nc.scalar.activation(ex[:], m2b[:], AF.Exp, bias=negmax[:], accum_out=ssum[:])
                rec = gm.tile([128, 1], F32, tag="rec")
                nc.vector.reciprocal(rec[:], ssum[:])
                smt = gm.tile([128, B], F32, tag="smt")
                nc.vector.tensor_scalar_mul(smt[:], ex[:], rec[:])
                nc.sync.dma_start(sm_out[:], smt[:])

    nc.compile()
    return nc


def _sbuf_layout(lhsT):
    """[K, M] -> [128, (K/128)*M] host layout (k-tiles side by side)."""
    K, M = lhsT.shape
    assert K % 128 == 0
    return np.ascontiguousarray(
        lhsT.reshape(K // 128, 128, M).transpose(1, 0, 2).reshape(128, (K // 128) * M))


def _prep_inputs(inputs):
    f32 = np.float32
    W_ih1 = np.asarray(inputs["W_ih1"], f32); W_hh1 = np.asarray(inputs["W_hh1"], f32)
    W_ih2 = np.asarray(inputs["W_ih2"], f32); W_hh2 = np.asarray(inputs["W_hh2"], f32)
    W1 = np.asarray(inputs["W1"], f32); W2 = np.asarray(inputs["W2"], f32)
    b1 = np.asarray(inputs["b_ih1"], f32) + np.asarray(inputs["b_hh1"], f32)
    b2 = np.asarray(inputs["b_ih2"], f32) + np.asarray(inputs["b_hh2"], f32)
    bw1 = np.asarray(inputs["b1"], f32); bw2 = np.asarray(inputs["b2"], f32)
    X = np.asarray(inputs["input"], f32)          # (B, T, D)
    hidx = np.asarray(inputs["h"])                # (B, 5) int

    def pad_gate(Wm, kin, kin_p):
        """(4000, kin) -> (4096, kin_p) in padded gate space."""
        out = np.zeros((4, Hp, kin_p), f32)
        out[:, :H, :kin] = Wm.reshape(4, H, kin)
        return out.reshape(4 * Hp, kin_p)

    A1 = pad_gate(W_ih1[:, :D], D, Dp)
    B1 = pad_gate(W_ih1[:, D:] + W_hh1, H, Hp)
    A2 = pad_gate(W_ih2[:, D:D + H], H, Hp)
    B2 = pad_gate(W_ih2[:, D + H:] + W_hh2, H, Hp)

    # W1 in per-core-chunk padded mid space; columns [h1 | topic] each padded
    W1p = np.zeros((MIDp, 2 * Hp), f32)
    for k in range(N_CORES):
        W1p[k * 512:k * 512 + 500, :H] = W1[k * 500:(k + 1) * 500, :H]
        W1p[k * 512:k * 512 + 500, Hp:Hp + H] = W1[k * 500:(k + 1) * 500, H:]
    W2p = np.zeros((OUTp, MIDp), f32)
    for k in range(N_CORES):
        W2p[:OUT, k * 512:k * 512 + 500] = W2[:, k * 500:(k + 1) * 500]

    b1p = np.zeros((4, Hp), f32); b1p[:, :H] = b1.reshape(4, H)
    b2p = np.zeros((4, Hp), f32); b2p[:, :H] = b2.reshape(4, H)
    bw1p = np.zeros(MIDp, f32)
    for k in range(N_CORES):
        bw1p[k * 512:k * 512 + 500] = bw1[k * 500:(k + 1) * 500]
    bw2p = np.zeros(OUTp, f32); bw2p[:OUT] = bw2

    # x in device layout: (T, 128, KX*B) f16, replicated
    Xt = np.zeros((T, Dp, B), f32)
    Xt[:, :D, :] = X.transpose(1, 2, 0)
    xdev = np.ascontiguousarray(
        Xt.reshape(T, KX, 128, B).transpose(0, 2, 1, 3).reshape(T, 128, KX * B)
    ).astype(np.float16)

    topic = np.zeros((Hp, B), f32)
    topic[hidx.T % Hp, np.arange(B)[None, :]] = 1.0
    tp = topic.copy(); tp[H:] = 0.0
    topict = _sbuf_layout(tp).astype(np.float16)

    in_maps = []
    for k in range(N_CORES):
        rows = np.concatenate([np.arange(g * Hp + 128 * k, g * Hp + 128 * (k + 1))
                               for g in range(4)])
        d = {
            "xdev": xdev,
            "topict": topict,
            "a1w": _sbuf_layout(np.ascontiguousarray(A1[rows].T)).astype(np.float16),
            "b1w": _sbuf_layout(np.ascontiguousarray(B1[rows].T)).astype(np.float16),
            "a2w": _sbuf_layout(np.ascontiguousarray(A2[rows].T)).astype(np.float16),
            "b2w": _sbuf_layout(np.ascontiguousarray(B2[rows].T)).astype(np.float16),
            "w1w": _sbuf_layout(np.ascontiguousarray(W1p[k * 512:(k + 1) * 512].T)).astype(np.float16),
            "w2w": _sbuf_layout(np.ascontiguousarray(W2p[:, k * 512:(k + 1) * 512].T)).astype(np.float16),
            "b1c": np.ascontiguousarray(b1p[:, 128 * k:128 * (k + 1)].T),
            "b2c": np.ascontiguousarray(b2p[:, 128 * k:128 * (k + 1)].T),
            "bh1": np.ascontiguousarray(bw1p[k * 512:(k + 1) * 512].reshape(4, 128).T),
            "bh2": np.ascontiguousarray(bw2p[128 * k:128 * (k + 1)].reshape(128, 1)),
        }
        in_maps.append(d)
    return in_maps


def kernel(trace=False, tmpdir=None, **inputs):
    if "nc" not in _CACHE:
        _CACHE["nc"] = _build_nc()
    nc = _CACHE["nc"]
    in_maps = _prep_inputs(inputs)
    kw = {}
    if trace:
        kw = dict(trace=True, tmpdir=tmpdir)
    r = bass_utils.run_bass_kernel_spmd(nc, in_maps, core_ids=list(range(N_CORES)), **kw)
    _CACHE["last_results"] = r
    res = r.results

    sm = np.concatenate([res[k]["sm"] for k in range(N_CORES)], axis=0)
    h1 = np.concatenate([res[k]["h1c"] for k in range(N_CORES)], axis=0).astype(np.float32)
    c1 = np.concatenate([res[k]["c1c"] for k in range(N_CORES)], axis=0)
    h2 = np.concatenate([res[k]["h2c"] for k in range(N_CORES)], axis=0).astype(np.float32)
    c2 = np.concatenate([res[k]["c2c"] for k in range(N_CORES)], axis=0)

    softmax = np.ascontiguousarray(sm[:OUT].T)
    return (softmax,
            (np.ascontiguousarray(h1[:H].T), np.ascontiguousarray(c1[:H].T)),
            (np.ascontiguousarray(h2[:H].T), np.ascontiguousarray(c2[:H].T)))
